# revision 1
# baseline (speedup 1.0000x reference)
"""DistanceLoss kernel for Trainium2 (8 NeuronCores, data-parallel over batch).

Computes mean(MARGIN + dist[i, label_i] - min_{c != label_i} dist[i, c]) where
dist is the pairwise L2 distance between row-normalized WO [N, D] and class
embeddings emb [C, D], via the GEMM identity d2 = x2 + e2 - 2 * WOn @ emb.T.

Per core (2048 rows): PSUM = 2*An@E.T - e2 (fp8e4 DoubleRow matmuls, e2 as an
exact fp16 hi/lo rank-2 matmul), so d2 = 1 - psum. The min over classes !=
label is one custom-DVE TENSOR_MASK_REDUCE per psum tile using an inverted
per-row single-index window (start = col+1 > end = col selects everything
except the label's column) with the two halves chained through the accum
init - exact masked min in a single scan. The label distance goes through a
full-f32 path (indirect-DMA row gather of emb[label], fused multiply-reduce
dot, ScalarE square-accumulate), so matmul quantization never touches it.
rsqrt/sqrt run on DVE via bit-trick seed + Newton steps, keeping ScalarE on a
single LUT table set (Square/Copy) with no table-switch stalls.

Layout tricks: row-block m holds rows {i : i % 16 == m} and class-block c
holds classes {j : j % 16 == c}, which makes every DMA (WO, emb, labels)
contiguous per partition (few large descriptors - descriptor generation, not
bandwidth, dominates DGE cost), at the price of a cheap exact bit-op remap of
the label's matrix column. Loads are split across the SP/Activation HWDGE
queues and issued ahead of any compute on those sequencers; transposes,
GEMMs, and reductions are emission-interleaved so no in-order engine queue
ever convoys behind a late producer.

End-to-end relative error vs the f32 reference: ~3.1e-6 (measured on HW).
Sharding: WO/label split over N across 8 cores, emb replicated; mean on host.
"""

import sys

if "/opt/trn_rl_repo" not in sys.path:
    sys.path.insert(0, "/opt/trn_rl_repo")

import numpy as np

import concourse.bacc as bacc
import concourse.bass as bass
import concourse.mybir as mybir
import concourse.tile as tile
from concourse.bass_utils import run_bass_kernel_spmd
from concourse.dve_ops import TENSOR_MASK_REDUCE, TENSOR_TENSOR_REDUCE
from concourse.masks import make_identity

MARGIN = 1.0
N_CORES = 8
N_FULL, C, D = 16384, 2048, 512
P = 128
NN = N_FULL // N_CORES          # rows per core (2048)
NT = NN // P                    # row tiles per core (16)
CT = C // P                     # class tiles (16)
KT = D // P                     # contraction tiles (4)
HALF = C // 2                   # psum tile width (1024)

f32 = mybir.dt.float32
f16 = mybir.dt.float16
f8 = mybir.dt.float8e4
i32 = mybir.dt.int32
FP8 = True  # fp8e4 DoubleRow main matmuls (measured end-to-end ~1e-6 rel err)
Alu = mybir.AluOpType
Act = mybir.ActivationFunctionType

NEG_BIG = -3.0e38
QUAKE = 0x5F3759DF


def _rsqrt(nc, pool, x_ap, w, name, iters=3):
    """1/sqrt(x) on DVE: bit-trick seed + Newton. x_ap: [P, w] f32."""
    si = pool.tile([P, w], i32, tag=f"rs_i{name}")
    nc.vector.tensor_scalar(
        out=si[:], in0=x_ap.bitcast(i32), scalar1=1, scalar2=0,
        op0=Alu.logical_shift_right, op1=Alu.bitwise_not,
    )
    nc.vector.tensor_scalar(out=si[:], in0=si[:], scalar1=QUAKE + 1, scalar2=None,
                            op0=Alu.add)
    y = pool.tile([P, w], f32, tag=f"rs_y{name}")
    nc.vector.tensor_copy(out=y[:], in_=si[:].bitcast(f32))
    t = pool.tile([P, w], f32, tag=f"rs_t{name}")
    for _ in range(iters):
        nc.vector.tensor_mul(out=t[:], in0=y[:], in1=y[:])
        nc.vector.tensor_mul(out=t[:], in0=t[:], in1=x_ap)
        nc.vector.tensor_scalar(out=t[:], in0=t[:], scalar1=-0.5, scalar2=1.5,
                                op0=Alu.mult, op1=Alu.add)
        nc.vector.tensor_mul(out=y[:], in0=y[:], in1=t[:])
    return y


def _build():
    nc = bacc.Bacc("TRN2", target_bir_lowering=False, debug=False)

    wo_d = nc.dram_tensor("WO", [NN, D], f32, kind="ExternalInput")
    emb_d = nc.dram_tensor("emb", [C, D], f32, kind="ExternalInput")
    lab_d = nc.dram_tensor("label", [NN, 1], i32, kind="ExternalInput")
    out_d = nc.dram_tensor("out", [P, NT], f32, kind="ExternalOutput")

    with tile.TileContext(nc) as tc:
        with (
            tc.tile_pool(name="persist", bufs=1) as pp,
            tc.tile_pool(name="an", bufs=NT) as anp,
            tc.tile_pool(name="ex", bufs=CT) as exp_,
            tc.tile_pool(name="elab", bufs=NT) as elp,
            tc.tile_pool(name="sq", bufs=2) as sqp,
            tc.tile_pool(name="tmp", bufs=8) as tmp_p,
            tc.tile_pool(name="mm", bufs=2, space="PSUM") as mmp,
            tc.tile_pool(name="tp", bufs=4, space="PSUM") as tpp,
        ):
            # ---- constants ----
            ident = pp.tile([P, P], f16)
            make_identity(nc, ident[:])
            identf = pp.tile([P, P], f32)
            make_identity(nc, identf[:])
            ones2 = pp.tile([2, P], f16)
            nc.vector.memset(ones2[:], 1.0)

            # ---- interleaved E + WO pipelines ----
            # Row-block m covers rows {i : i % NT == m} and class-block c covers
            # classes {j : j % CT == c}: partition p's 16 rows are contiguous
            # 32KB in DRAM, so a whole group-of-4 loads as one DMA with 8KB
            # descriptors (descriptor generation, not bandwidth, is the DMA
            # bottleneck).  Matrix column jc = c*128 + p holds class p*16 + c.
            # E tile c: load -> ACT square (e2 col) -> DVE cast (2E fp16)
            # WO tile t: load -> ACT square (x2) -> rnorm (group of 4)
            #            -> DVE cast (An fp16)
            e2c = pp.tile([P, CT], f32)
            x2 = pp.tile([P, NT], f32)
            an = []
            rnorm = pp.tile([P, NT], f32)
            dump = pp.tile([P, 1], f32)
            e2s_dram = nc.dram_tensor("e2scratch", [2, C], f16)
            e2pair = pp.tile([2, C], f16)
            mm_dt = f8 if FP8 else f16
            eT = pp.tile([P, KT, C], mm_dt)
            aT = pp.tile([P, KT, NN], mm_dt)
            e_all = pp.tile([P, CT, D], f32)
            wo_all = pp.tile([P, NT, D], f32)
            emb_v = emb_d.rearrange("(p c) d -> p c d", c=CT)
            wo_v = wo_d.rearrange("(p t) d -> p t d", t=NT)

            # labels first on the Pool queue; group loads up-front so neither
            # HWDGE queue ever waits behind compute issued from the same SEQ
            labi = pp.tile([P, NT], i32)
            nc.gpsimd.dma_start(
                out=labi[:], in_=lab_d[:, 0].rearrange("(p m) -> p m", m=NT))
            for g in range(4):
                sl = slice(g * 4, (g + 1) * 4)
                nc.sync.dma_start(out=e_all[:, sl, :], in_=emb_v[:, sl, :])
                nc.scalar.dma_start(out=wo_all[:, sl, :], in_=wo_v[:, sl, :])

            negmax = pp.tile([P, NT], f32)
            acc0 = pp.tile([P, NT], f32)
            dots = pp.tile([P, NT], f32)
            elab2 = pp.tile([P, NT], f32)
            elab_tiles = [None] * NT
            ex = [None] * CT

            def prep_group(g):
                sl = slice(g * 4, (g + 1) * 4)
                for t in range(g * 4, (g + 1) * 4):
                    s = sqp.tile([P, D], f16, tag="sq", name=f"sq_{t}")
                    nc.scalar.activation(out=s[:], in_=e_all[:, t, :], func=Act.Square,
                                         accum_out=e2c[:, t : t + 1])
                    sw = sqp.tile([P, D], f16, tag="sq", name=f"sqw_{t}")
                    nc.scalar.activation(out=sw[:], in_=wo_all[:, t, :], func=Act.Square,
                                         accum_out=x2[:, t : t + 1])
                # -e2 quarter as fp16 hi/lo rows (DRAM bounce for the
                # partition->free transpose; matrix columns g*512..g*512+511)
                e2n = tmp_p.tile([P, 4], f32, tag="e2n", name=f"e2n_{g}")
                nc.vector.tensor_scalar_mul(out=e2n[:], in0=e2c[:, sl], scalar1=-1.0)
                e2hi = tmp_p.tile([P, 4], f16, tag="e2hi", name=f"e2hi_{g}")
                nc.vector.tensor_copy(out=e2hi[:], in_=e2n[:])
                e2hf = tmp_p.tile([P, 4], f32, tag="e2hf", name=f"e2hf_{g}")
                nc.vector.tensor_copy(out=e2hf[:], in_=e2hi[:])
                e2lo = tmp_p.tile([P, 4], f32, tag="e2lo", name=f"e2lo_{g}")
                nc.vector.tensor_sub(out=e2lo[:], in0=e2n[:], in1=e2hf[:])
                e2lo16 = tmp_p.tile([P, 4], f16, tag="e2lo16", name=f"e2lo16_{g}")
                nc.vector.tensor_copy(out=e2lo16[:], in_=e2lo[:])
                qs = slice(g * 512, (g + 1) * 512)
                nc.sync.dma_start(
                    out=e2s_dram[0:1, qs].rearrange("o (ct p) -> o p ct", p=P),
                    in_=e2hi[:])
                nc.sync.dma_start(
                    out=e2s_dram[1:2, qs].rearrange("o (ct p) -> o p ct", p=P),
                    in_=e2lo16[:])
                nc.sync.dma_start(out=e2pair[:, qs], in_=e2s_dram[:, qs])
                # rnorm + An casts for WO tiles g*4..g*4+3
                y = _rsqrt(nc, tmp_p, x2[:, sl], 4, "n", iters=2)
                nc.vector.tensor_scalar_min(out=rnorm[:, sl], in0=y[:], scalar1=1.0e12)
                for tt in range(g * 4, (g + 1) * 4):
                    a = anp.tile([P, D], f16, tag="an", name=f"an_{tt}")
                    an.append(a)
                    nc.vector.tensor_scalar_mul(out=a[:], in0=wo_all[:, tt, :],
                                                scalar1=rnorm[:, tt : tt + 1])
                # transposes for this group of 4 (E then A)
                for cc in range(g * 4, (g + 1) * 4):
                    # transpose straight from f32 (skips a cast hop); the *2
                    # scale and fp8 cast ride the PSUM->SBUF copy
                    tp = tpp.tile([P, KT, P], f32, tag="tp", name=f"tpe_{cc}")
                    for k in range(KT):
                        nc.tensor.transpose(out=tp[:, k, :],
                                            in_=e_all[:, cc, k * P : (k + 1) * P],
                                            identity=identf[:])
                    if cc % 2 == 0:
                        nc.scalar.activation(out=eT[:, :, cc * P : (cc + 1) * P],
                                             in_=tp[:], func=Act.Copy, scale=2.0)
                    else:
                        nc.vector.tensor_scalar_mul(
                            out=eT[:, :, cc * P : (cc + 1) * P], in0=tp[:],
                            scalar1=2.0)
                for mm in range(g * 4, (g + 1) * 4):
                    tp = tpp.tile([P, KT, P], f16, tag="tp", name=f"tpa_{mm}")
                    for k in range(KT):
                        nc.tensor.transpose(out=tp[:, k, :],
                                            in_=an[mm][:, k * P : (k + 1) * P],
                                            identity=ident[:])
                    if mm % 2 == 1:
                        nc.scalar.copy(out=aT[:, :, mm * P : (mm + 1) * P], in_=tp[:])
                    else:
                        nc.vector.tensor_copy(out=aT[:, :, mm * P : (mm + 1) * P], in_=tp[:])

            pm_tiles = {}

            def mm_mms(h, m):
                pm = mmp.tile([P, HALF], f32, tag="mm", name=f"pm_{h}_{m}")
                pm_tiles[(h, m)] = pm
                for ns in range(2):
                    col0 = h * HALF + ns * 512
                    if FP8:
                        for kp in range(0, KT, 2):
                            nc.tensor.matmul(
                                out=pm[:, ns * 512 : (ns + 1) * 512],
                                lhsT=aT[:, kp : kp + 2, m * P : (m + 1) * P],
                                rhs=eT[:, kp : kp + 2, col0 : col0 + 512],
                                start=(kp == 0), stop=False,
                                perf_mode=mybir.MatmulPerfMode.DoubleRow,
                            )
                    else:
                        for k in range(KT):
                            nc.tensor.matmul(
                                out=pm[:, ns * 512 : (ns + 1) * 512],
                                lhsT=aT[:, k, m * P : (m + 1) * P],
                                rhs=eT[:, k, col0 : col0 + 512],
                                start=(k == 0), stop=False,
                            )
                    nc.tensor.matmul(
                        out=pm[:, ns * 512 : (ns + 1) * 512],
                        lhsT=ones2[:], rhs=e2pair[:, col0 : col0 + 512],
                        start=False, stop=True,
                    )
            def mm_red(h, m):
                st_all = labf1 if h == 0 else labh1
                en_all = labj if h == 0 else labh
                pm = pm_tiles[(h, m)]
                # masked max over c != label (inverted single-index window)
                dmp = tmp_p.tile([P, 1], f32, tag="dmp", name=f"dmp_{h}_{m}")
                nc.vector._custom_dve(
                    TENSOR_MASK_REDUCE,
                    out=dmp[:].broadcast_to([P, HALF]),
                    in0=pm[:],
                    in1=en_all[:, m : m + 1],
                    s0=st_all[:, m : m + 1],
                    s1=NEG_BIG if h == 0 else acc0[:, m : m + 1],
                    imm2=1.0,
                    accum_out=(acc0 if h == 0 else negmax)[:, m : m + 1],
                )
                if h == 0:
                    # label path (f32): gather emb[label] (Pool SEQ descriptor
                    # generation is the gather bottleneck -> start early)
                    g = elp.tile([P, D], f32, tag="elab", name=f"elab_{m}")
                    elab_tiles[m] = g
                    nc.gpsimd.indirect_dma_start(
                        out=g[:], out_offset=None, in_=emb_d[:, :],
                        in_offset=bass.IndirectOffsetOnAxis(
                            ap=labi[:, m : m + 1], axis=0),
                    )
                    s = sqp.tile([P, D], f16, tag="sq", name=f"sql_{m}")
                    nc.scalar.activation(out=s[:], in_=g[:], func=Act.Square,
                                         accum_out=elab2[:, m : m + 1])
                # dots split across both passes to balance DVE per-m load
                if (h == 0 and m % 2 == 0) or (h == 1 and m % 2 == 1):
                    dmp2 = tmp_p.tile([P, 1], f32, tag="dmp", name=f"dmpd_{h}_{m}")
                    nc.vector._custom_dve(
                        TENSOR_TENSOR_REDUCE, out=dmp2[:].broadcast_to([P, D]),
                        in0=wo_all[:, m, :], in1=elab_tiles[m][:], s0=0.0, s1=1.0,
                        accum_out=dots[:, m : m + 1],
                    )

            # pipeline: h0 GEMMs for a group start as soon as eT[0:1024]
            # (groups 0-1) and that group's aT exist; h1 after all transposes
            prep_group(0)
            prep_group(1)
            for m in range(0, 4):
                mm_mms(0, m)
            prep_group(2)
            for m in range(4, 8):
                mm_mms(0, m)
            prep_group(3)
            for m in range(8, 16):
                mm_mms(0, m)
            # label window coordinates: block-major layout [p, m] =
            # label[p*NT + m]; matrix column of class L is (L % CT)*P + L // CT
            # col = (label & 15) << 7 | (label >> 4), in exact int bit ops
            lm = tmp_p.tile([P, NT], i32, tag="lm")
            nc.vector.tensor_scalar(out=lm[:], in0=labi[:], scalar1=15, scalar2=7,
                                    op0=Alu.bitwise_and, op1=Alu.logical_shift_left)
            ld = tmp_p.tile([P, NT], i32, tag="ld")
            nc.vector.tensor_scalar(out=ld[:], in0=labi[:], scalar1=4, scalar2=None,
                                    op0=Alu.logical_shift_right)
            nc.vector.tensor_tensor(out=lm[:], in0=lm[:], in1=ld[:], op=Alu.bitwise_or)
            labj = pp.tile([P, NT], f32)       # column index of label class
            nc.vector.tensor_copy(out=labj[:], in_=lm[:])
            labf1 = pp.tile([P, NT], f32)      # col + 1
            nc.vector.tensor_scalar_add(out=labf1[:], in0=labj[:], scalar1=1.0)
            labh = pp.tile([P, NT], f32)       # col - HALF
            nc.vector.tensor_scalar_add(out=labh[:], in0=labj[:], scalar1=float(-HALF))
            labh1 = pp.tile([P, NT], f32)      # col - HALF + 1
            nc.vector.tensor_scalar_add(out=labh1[:], in0=labj[:], scalar1=float(1 - HALF))

            for m in range(16):
                mm_red(0, m)
            for m in range(16):
                mm_mms(1, m)
                mm_red(1, m)

            # ---- epilogue ----
            # label_d2 = 1 + elab2 - 2*rnorm*dot  (x2 of normalized row == 1)
            ld2 = tmp_p.tile([P, NT], f32, tag="ld2")
            nc.vector.tensor_mul(out=ld2[:], in0=rnorm[:], in1=dots[:])
            nc.vector.tensor_scalar(out=ld2[:], in0=ld2[:], scalar1=-2.0, scalar2=1.0,
                                    op0=Alu.mult, op1=Alu.add)
            nc.vector.tensor_add(out=ld2[:], in0=ld2[:], in1=elab2[:])
            nc.vector.tensor_scalar_max(out=ld2[:], in0=ld2[:], scalar1=0.0)
            # min_{c!=lab} d2 = 1 - negmax
            md2 = tmp_p.tile([P, NT], f32, tag="md2")
            nc.vector.tensor_scalar(out=md2[:], in0=negmax[:], scalar1=-1.0, scalar2=1.0,
                                    op0=Alu.mult, op1=Alu.add)
            nc.vector.tensor_scalar_max(out=md2[:], in0=md2[:], scalar1=0.0)

            # sqrt(x) = x * rsqrt(x); out = sqrt(ld2) - sqrt(md2)
            rl = _rsqrt(nc, tmp_p, ld2[:], NT, "l")
            rm = _rsqrt(nc, tmp_p, md2[:], NT, "m")
            nc.vector.tensor_mul(out=rl[:], in0=rl[:], in1=ld2[:])
            nc.vector.tensor_mul(out=rm[:], in0=rm[:], in1=md2[:])
            outv = pp.tile([P, NT], f32)
            nc.vector.tensor_sub(out=outv[:], in0=rl[:], in1=rm[:])
            nc.gpsimd.dma_start(out=out_d[:, :], in_=outv[:])

    nc.compile()
    return nc


_NC = None


def kernel(WO, emb_weight, label):
    global _NC
    if _NC is None:
        _NC = _build()

    WO = np.ascontiguousarray(np.asarray(WO, dtype=np.float32))
    emb = np.ascontiguousarray(np.asarray(emb_weight, dtype=np.float32))
    lab = np.asarray(label).astype(np.int32).reshape(N_FULL, 1)

    in_maps = []
    for i in range(N_CORES):
        sl = slice(i * NN, (i + 1) * NN)
        in_maps.append({
            "WO": WO[sl],
            "emb": emb,
            "label": np.ascontiguousarray(lab[sl]),
        })
    res = run_bass_kernel_spmd(_NC, in_maps, core_ids=list(range(N_CORES)))
    vals = np.stack([res.results[i]["out"] for i in range(N_CORES)])
    return np.float32(MARGIN + np.mean(vals.astype(np.float64)))



# revision 23
# speedup vs baseline: 27.5371x; 27.5371x over previous
"""DistanceLoss kernel for Trainium2 (8 NeuronCores, data-parallel over batch).

Computes mean(MARGIN + dist[i, label_i] - min_{c != label_i} dist[i, c]) where
dist is the pairwise L2 distance between row-normalized WO [N, D] and class
embeddings emb [C, D], via the GEMM identity d2 = x2 + e2 - 2 x.e.

Design (per core, 2048 rows), measured at ~149us on HW via neuron-profile
(baseline before this optimization round: ~227us):
- Inputs are host-cast to bf16 (halves HBM traffic; measured end-to-end rel
  err ~1.2e-3 vs the f32 reference, 16x under the 2e-2 gate).
- The GEMM runs on RAW wo (no row-normalize pass): PSUM = 2*wo@E.T - s*e2
  where s = |wo| rides as an f16 hi/lo pair in a rank-4 matmul
  (lhsT=[shi,shi,slo,slo], rhs=[-e2hi,-e2lo,-e2hi,-e2lo]) so the e2 term is
  row-scaled exactly; the epilogue rescales by rnorm = 1/s. Main matmuls are
  fp8e4 DoubleRow; eT carries the x2 scale (cast with scale=2).
- e2/s never bounce through DRAM: per load-group the f16 hi/lo columns are
  stacked in one [128, 16] staging tile, PE-transposed once, and scattered to
  the [4, C] quad tiles with 4-descriptor SBUF->SBUF DMAs on the Pool queue.
- Loads are spread over all three DMA queues (sync: e groups 0-2, scalar:
  wo groups 0-2, gpsimd: group 3 of both), ~100 GB/s per HWDGE queue, so the
  first GEMM chain starts at ~25us instead of ~95us.
- Masked min over classes != label is a chained TENSOR_MASK_REDUCE scan
  (inverted per-row single-index window: start = col+1 > end = col selects
  everything except the label's column; the two [128,1024] halves chain
  through the accum init). Label distance: indirect-DMA row gather of
  emb[label] (bf16), fused multiply-reduce dot, ScalarE square-accumulate,
  all off the critical path.
- Row-block/class-block layout: row p*16+t lives at partition p, tile t, so
  every big DMA is 128 descriptors of contiguous 4KB; the label's matrix
  column is an exact bit-op remap col = (L & 15) << 7 | (L >> 4).

Things measured NOT to help (kept out): DoublePixel/DoubleColumn perf modes
(silently ignored by the compiler), 1024-wide matmul chains (ISA check caps
PSUM writes at 512 f32/partition), moving squares to DVE or staging to Pool
(starves the scan stream / convoys the in-order queues and resets the PE
p-state ramp, making every matmul ~1.5x slower), fp8 inputs everywhere
(1.77e-2 rel err - too close to the 2e-2 gate).

Sharding: WO/label split over N across 8 cores, emb replicated; mean on host.
"""

import sys

if "/opt/trn_rl_repo" not in sys.path:
    sys.path.insert(0, "/opt/trn_rl_repo")

import numpy as np

import concourse.bacc as bacc
import concourse.bass as bass
import concourse.mybir as mybir
import concourse.tile as tile
from concourse.bass_utils import run_bass_kernel_spmd
from concourse.dve_ops import TENSOR_MASK_REDUCE, TENSOR_TENSOR_REDUCE
from concourse.masks import make_identity

MARGIN = 1.0
N_CORES = 8
N_FULL, C, D = 16384, 2048, 512
P = 128
NN = N_FULL // N_CORES          # rows per core (2048)
NT = NN // P                    # row tiles per core (16)
CT = C // P                     # class tiles (16)
KT = D // P                     # contraction tiles (4)
HALF = C // 2                   # psum tile width (1024)

f32 = mybir.dt.float32
f16 = mybir.dt.float16
bf16 = mybir.dt.bfloat16
f8 = mybir.dt.float8e4
i32 = mybir.dt.int32
Alu = mybir.AluOpType
Act = mybir.ActivationFunctionType

NEG_BIG = -3.0e38
QUAKE = 0x5F3759DF


def _rsqrt(nc, pool, x_ap, w, name, iters=3):
    """1/sqrt(x) on DVE: bit-trick seed + Newton. x_ap: [P, w] f32."""
    si = pool.tile([P, w], i32, tag=f"rs_i{name}")
    nc.vector.tensor_scalar(
        out=si[:], in0=x_ap.bitcast(i32), scalar1=1, scalar2=0,
        op0=Alu.logical_shift_right, op1=Alu.bitwise_not,
    )
    nc.vector.tensor_scalar(out=si[:], in0=si[:], scalar1=QUAKE + 1, scalar2=None,
                            op0=Alu.add)
    y = pool.tile([P, w], f32, tag=f"rs_y{name}")
    nc.vector.tensor_copy(out=y[:], in_=si[:].bitcast(f32))
    t = pool.tile([P, w], f32, tag=f"rs_t{name}")
    for _ in range(iters):
        nc.vector.tensor_mul(out=t[:], in0=y[:], in1=y[:])
        nc.vector.tensor_mul(out=t[:], in0=t[:], in1=x_ap)
        nc.vector.tensor_scalar(out=t[:], in0=t[:], scalar1=-0.5, scalar2=1.5,
                                op0=Alu.mult, op1=Alu.add)
        nc.vector.tensor_mul(out=y[:], in0=y[:], in1=t[:])
    return y


def _build(debug_taps=False):
    nc = bacc.Bacc("TRN2", target_bir_lowering=False, debug=False)

    wo_d = nc.dram_tensor("WO", [NN, D], bf16, kind="ExternalInput")
    emb_d = nc.dram_tensor("emb", [C, D], bf16, kind="ExternalInput")
    lab_d = nc.dram_tensor("label", [NN, 1], i32, kind="ExternalInput")
    out_d = nc.dram_tensor("out", [P, NT], f32, kind="ExternalOutput")

    with tile.TileContext(nc) as tc:
        with (
            tc.tile_pool(name="persist", bufs=1) as pp,
            tc.tile_pool(name="elab", bufs=NT) as elp,
            tc.tile_pool(name="sq", bufs=2) as sqp,
            tc.tile_pool(name="tmp", bufs=8) as tmp_p,
            tc.tile_pool(name="mm", bufs=2, space="PSUM") as mmp,
            tc.tile_pool(name="tp", bufs=4, space="PSUM") as tpp,
        ):
            # ---- constants ----
            ident_b = pp.tile([P, P], bf16)
            make_identity(nc, ident_b[:])
            ident_h = pp.tile([P, P], f16)
            make_identity(nc, ident_h[:])

            e2c = pp.tile([P, CT], f32)
            x2 = pp.tile([P, NT], f32)
            rnorm = pp.tile([P, NT], f32)
            eT = pp.tile([P, KT, C], f8)
            aT = pp.tile([P, KT, NN], f8)
            wquad = pp.tile([4, NN], f16)   # rows shi,shi,slo,slo (cols = rows)
            e2quad = pp.tile([4, C], f16)   # rows -e2hi,-e2lo,-e2hi,-e2lo
            e_all = pp.tile([P, CT, D], bf16)
            wo_all = pp.tile([P, NT, D], bf16)
            emb_v = emb_d.rearrange("(p c) d -> p c d", c=CT)
            wo_v = wo_d.rearrange("(p t) d -> p t d", t=NT)

            # ---- loads: 3 queues in parallel ----
            labi = pp.tile([P, NT], i32)
            nc.gpsimd.dma_start(
                out=labi[:], in_=lab_d[:, 0].rearrange("(p m) -> p m", m=NT))
            for g in range(3):
                sl = slice(g * 4, (g + 1) * 4)
                nc.sync.dma_start(out=e_all[:, sl, :], in_=emb_v[:, sl, :])
                nc.scalar.dma_start(out=wo_all[:, sl, :], in_=wo_v[:, sl, :])
            sl3 = slice(12, 16)
            nc.gpsimd.dma_start(out=e_all[:, sl3, :], in_=emb_v[:, sl3, :])
            nc.gpsimd.dma_start(out=wo_all[:, sl3, :], in_=wo_v[:, sl3, :])

            negmax = pp.tile([P, NT], f32)
            acc0 = pp.tile([P, NT], f32)
            dots = pp.tile([P, NT], f32)
            elab2 = pp.tile([P, NT], f32)
            elab_tiles = [None] * NT

            def prep_group(g):
                sl = slice(g * 4, (g + 1) * 4)
                for t in range(g * 4, (g + 1) * 4):
                    s = sqp.tile([P, D], f16, tag="sq", name=f"sq_{t}")
                    nc.scalar.activation(out=s[:], in_=e_all[:, t, :],
                                         func=Act.Square,
                                         accum_out=e2c[:, t : t + 1])
                    sw = sqp.tile([P, D], f16, tag="sq", name=f"sqw_{t}")
                    nc.scalar.activation(out=sw[:], in_=wo_all[:, t, :],
                                         func=Act.Square,
                                         accum_out=x2[:, t : t + 1])
                # s = |wo| and -e2, each split into f16 hi/lo columns of one
                # [P, 16] staging tile: cols 0-3 shi, 4-7 slo, 8-11 e2hi,
                # 12-15 e2lo (for row/class tiles g*4..g*4+3)
                y = _rsqrt(nc, tmp_p, x2[:, sl], 4, "n", iters=3)
                nc.vector.tensor_scalar_min(out=rnorm[:, sl], in0=y[:], scalar1=1.0e12)
                s_ = tmp_p.tile([P, 4], f32, tag="s_", name=f"s_{g}")
                nc.vector.tensor_mul(out=s_[:], in0=rnorm[:, sl], in1=x2[:, sl])
                stg = tmp_p.tile([P, 16], f16, tag="stg", name=f"stg_{g}")
                nc.vector.tensor_copy(out=stg[:, 0:4], in_=s_[:])
                hf = tmp_p.tile([P, 4], f32, tag="hf", name=f"hf_{g}")
                nc.vector.tensor_copy(out=hf[:], in_=stg[:, 0:4])
                lo = tmp_p.tile([P, 4], f32, tag="lo", name=f"lo_{g}")
                nc.vector.tensor_sub(out=lo[:], in0=s_[:], in1=hf[:])
                nc.vector.tensor_copy(out=stg[:, 4:8], in_=lo[:])
                e2n = tmp_p.tile([P, 4], f32, tag="e2n", name=f"e2n_{g}")
                nc.vector.tensor_scalar_mul(out=e2n[:], in0=e2c[:, sl], scalar1=-1.0)
                nc.vector.tensor_copy(out=stg[:, 8:12], in_=e2n[:])
                nc.vector.tensor_copy(out=hf[:], in_=stg[:, 8:12])
                nc.vector.tensor_sub(out=lo[:], in0=e2n[:], in1=hf[:])
                nc.vector.tensor_copy(out=stg[:, 12:16], in_=lo[:])
                # one tiny PE transpose -> [16, 128] psum, copy to SBUF, then
                # 4-descriptor Pool-queue DMAs scatter into the quad tiles
                tps = tpp.tile([P, KT, P], f16, tag="tp", name=f"tps_{g}")
                nc.tensor.transpose(out=tps[0:16, 0, :], in_=stg[:],
                                    identity=ident_h[:])
                stgT = tmp_p.tile([16, P], f16, tag="stgT", name=f"stgT_{g}")
                nc.vector.tensor_copy(out=stgT[:], in_=tps[0:16, 0, :])
                qs = slice(g * 512, (g + 1) * 512)
                for j in range(4):
                    src = stgT[0:4, :] if j < 2 else stgT[4:8, :]
                    nc.gpsimd.dma_start(out=wquad[j : j + 1, qs], in_=src)
                for j in range(4):
                    src = stgT[8:12, :] if j % 2 == 0 else stgT[12:16, :]
                    nc.gpsimd.dma_start(out=e2quad[j : j + 1, qs], in_=src)
                # transposes for this group of 4 (E then A), bf16 1 cyc/row
                for cc in range(g * 4, (g + 1) * 4):
                    tp = tpp.tile([P, KT, P], bf16, tag="tp", name=f"tpe_{cc}")
                    for k in range(KT):
                        nc.tensor.transpose(out=tp[:, k, :],
                                            in_=e_all[:, cc, k * P : (k + 1) * P],
                                            identity=ident_b[:])
                    if cc % 2 == 0:
                        nc.scalar.activation(out=eT[:, :, cc * P : (cc + 1) * P],
                                             in_=tp[:], func=Act.Copy, scale=2.0)
                    else:
                        nc.vector.tensor_scalar_mul(
                            out=eT[:, :, cc * P : (cc + 1) * P], in0=tp[:],
                            scalar1=2.0)
                for mm in range(g * 4, (g + 1) * 4):
                    tp = tpp.tile([P, KT, P], bf16, tag="tp", name=f"tpa_{mm}")
                    for k in range(KT):
                        nc.tensor.transpose(out=tp[:, k, :],
                                            in_=wo_all[:, mm, k * P : (k + 1) * P],
                                            identity=ident_b[:])
                    if mm % 2 == 1:
                        nc.scalar.copy(out=aT[:, :, mm * P : (mm + 1) * P], in_=tp[:])
                    else:
                        nc.vector.tensor_copy(out=aT[:, :, mm * P : (mm + 1) * P],
                                              in_=tp[:])

            pm_tiles = {}

            def mm_mms(h, m):
                pm = mmp.tile([P, HALF], f32, tag="mm", name=f"pm_{h}_{m}")
                pm_tiles[(h, m)] = pm
                for ns in range(2):
                    col0 = h * HALF + ns * 512
                    for kp in range(0, KT, 2):
                        nc.tensor.matmul(
                            out=pm[:, ns * 512 : (ns + 1) * 512],
                            lhsT=aT[:, kp : kp + 2, m * P : (m + 1) * P],
                            rhs=eT[:, kp : kp + 2, col0 : col0 + 512],
                            start=(kp == 0), stop=False,
                            perf_mode=mybir.MatmulPerfMode.DoubleRow,
                        )
                    nc.tensor.matmul(
                        out=pm[:, ns * 512 : (ns + 1) * 512],
                        lhsT=wquad[:, m * P : (m + 1) * P],
                        rhs=e2quad[:, col0 : col0 + 512],
                        start=False, stop=True,
                    )

            def mm_red(h, m):
                st_all = labf1 if h == 0 else labh1
                en_all = labj if h == 0 else labh
                pm = pm_tiles[(h, m)]
                # masked max over c != label (inverted single-index window)
                dmp = tmp_p.tile([P, 1], f32, tag="dmp", name=f"dmp_{h}_{m}")
                nc.vector._custom_dve(
                    TENSOR_MASK_REDUCE,
                    out=dmp[:].broadcast_to([P, HALF]),
                    in0=pm[:],
                    in1=en_all[:, m : m + 1],
                    s0=st_all[:, m : m + 1],
                    s1=NEG_BIG if h == 0 else acc0[:, m : m + 1],
                    imm2=1.0,
                    accum_out=(acc0 if h == 0 else negmax)[:, m : m + 1],
                )
                if h == 0:
                    # label path (bf16): gather emb[label] rows
                    g = elp.tile([P, D], bf16, tag="elab", name=f"elab_{m}")
                    elab_tiles[m] = g
                    nc.gpsimd.indirect_dma_start(
                        out=g[:], out_offset=None, in_=emb_d[:, :],
                        in_offset=bass.IndirectOffsetOnAxis(
                            ap=labi[:, m : m + 1], axis=0),
                    )
                    s = sqp.tile([P, D], f16, tag="sq", name=f"sql_{m}")
                    nc.scalar.activation(out=s[:], in_=g[:], func=Act.Square,
                                         accum_out=elab2[:, m : m + 1])
                # dots split across both passes to balance DVE per-m load
                if (h == 0 and m % 2 == 0) or (h == 1 and m % 2 == 1):
                    dmp2 = tmp_p.tile([P, 1], f32, tag="dmp", name=f"dmpd_{h}_{m}")
                    nc.vector._custom_dve(
                        TENSOR_TENSOR_REDUCE, out=dmp2[:].broadcast_to([P, D]),
                        in0=wo_all[:, m, :], in1=elab_tiles[m][:], s0=0.0, s1=1.0,
                        accum_out=dots[:, m : m + 1],
                    )

            # pipeline: h0 GEMMs for a group start as soon as eT[0:1024]
            # (groups 0-1) and that group's aT exist; h1 after all transposes
            prep_group(0)
            prep_group(1)
            for m in range(0, 4):
                mm_mms(0, m)
            prep_group(2)
            for m in range(4, 8):
                mm_mms(0, m)
            prep_group(3)
            for m in range(8, 16):
                mm_mms(0, m)
            # label window coordinates: block-major layout [p, m] =
            # label[p*NT + m]; matrix column of class L is (L % CT)*P + L // CT
            # col = (label & 15) << 7 | (label >> 4), in exact int bit ops
            lm = tmp_p.tile([P, NT], i32, tag="lm")
            nc.vector.tensor_scalar(out=lm[:], in0=labi[:], scalar1=15, scalar2=7,
                                    op0=Alu.bitwise_and, op1=Alu.logical_shift_left)
            ld = tmp_p.tile([P, NT], i32, tag="ld")
            nc.vector.tensor_scalar(out=ld[:], in0=labi[:], scalar1=4, scalar2=None,
                                    op0=Alu.logical_shift_right)
            nc.vector.tensor_tensor(out=lm[:], in0=lm[:], in1=ld[:], op=Alu.bitwise_or)
            labj = pp.tile([P, NT], f32)       # column index of label class
            nc.vector.tensor_copy(out=labj[:], in_=lm[:])
            labf1 = pp.tile([P, NT], f32)      # col + 1
            nc.vector.tensor_scalar_add(out=labf1[:], in0=labj[:], scalar1=1.0)
            labh = pp.tile([P, NT], f32)       # col - HALF
            nc.vector.tensor_scalar_add(out=labh[:], in0=labj[:], scalar1=float(-HALF))
            labh1 = pp.tile([P, NT], f32)      # col - HALF + 1
            nc.vector.tensor_scalar_add(out=labh1[:], in0=labj[:], scalar1=float(1 - HALF))

            for m in range(16):
                mm_red(0, m)
            for m in range(16):
                mm_mms(1, m)
                mm_red(1, m)

            # ---- epilogue ----
            # label_d2 = 1 + elab2 - 2*rnorm*dot  (x2 of normalized row == 1)
            ld2 = tmp_p.tile([P, NT], f32, tag="ld2")
            nc.vector.tensor_mul(out=ld2[:], in0=rnorm[:], in1=dots[:])
            nc.vector.tensor_scalar(out=ld2[:], in0=ld2[:], scalar1=-2.0, scalar2=1.0,
                                    op0=Alu.mult, op1=Alu.add)
            nc.vector.tensor_add(out=ld2[:], in0=ld2[:], in1=elab2[:])
            nc.vector.tensor_scalar_max(out=ld2[:], in0=ld2[:], scalar1=0.0)
            # min_{c!=lab} d2 = 1 - rnorm * negmax (psum scale is s = |wo|)
            md2 = tmp_p.tile([P, NT], f32, tag="md2")
            nc.vector.tensor_mul(out=md2[:], in0=rnorm[:], in1=negmax[:])
            nc.vector.tensor_scalar(out=md2[:], in0=md2[:], scalar1=-1.0, scalar2=1.0,
                                    op0=Alu.mult, op1=Alu.add)
            nc.vector.tensor_scalar_max(out=md2[:], in0=md2[:], scalar1=0.0)

            # sqrt(x) = x * rsqrt(x); out = sqrt(ld2) - sqrt(md2)
            rl = _rsqrt(nc, tmp_p, ld2[:], NT, "l")
            rm = _rsqrt(nc, tmp_p, md2[:], NT, "m")
            nc.vector.tensor_mul(out=rl[:], in0=rl[:], in1=ld2[:])
            nc.vector.tensor_mul(out=rm[:], in0=rm[:], in1=md2[:])
            outv = pp.tile([P, NT], f32)
            nc.vector.tensor_sub(out=outv[:], in0=rl[:], in1=rm[:])
            nc.gpsimd.dma_start(out=out_d[:, :], in_=outv[:])

            if debug_taps:
                tap_names = ["x2", "e2c", "rnorm", "negmax", "acc0", "dots",
                             "elab2", "labjT"]
                taps = {n: nc.dram_tensor(f"tap_{n}", [P, NT], f32,
                                          kind="ExternalOutput")
                        for n in tap_names}
                for name, src in [("x2", x2), ("e2c", e2c), ("rnorm", rnorm),
                                  ("negmax", negmax), ("acc0", acc0),
                                  ("dots", dots), ("elab2", elab2),
                                  ("labjT", labj)]:
                    nc.sync.dma_start(out=taps[name][:, :], in_=src[:])

    nc.compile()
    return nc


_NC = None


def kernel(WO, emb_weight, label):
    global _NC
    if _NC is None:
        _NC = _build()

    import ml_dtypes

    WO16 = np.ascontiguousarray(np.asarray(WO, dtype=np.float32)).astype(
        ml_dtypes.bfloat16)
    emb16 = np.ascontiguousarray(np.asarray(emb_weight, dtype=np.float32)).astype(
        ml_dtypes.bfloat16)
    lab = np.asarray(label).astype(np.int32).reshape(N_FULL, 1)

    in_maps = []
    for i in range(N_CORES):
        sl = slice(i * NN, (i + 1) * NN)
        in_maps.append({
            "WO": WO16[sl],
            "emb": emb16,
            "label": np.ascontiguousarray(lab[sl]),
        })
    res = run_bass_kernel_spmd(_NC, in_maps, core_ids=list(range(N_CORES)))
    vals = np.stack([res.results[i]["out"] for i in range(N_CORES)])
    return np.float32(MARGIN + np.mean(vals.astype(np.float64)))
